# revision 1
# baseline (speedup 1.0000x reference)
"""Trainium2 Bass kernel for the 4-layer quantized MLP (dense_mlp).

Strategy
--------
Data-parallel over the batch dim: each of the 8 cores processes 1024 of the
8192 rows with the full set of weights (no collectives; host gathers).

Numerics: quant_weight() yields integer codes q in {-3..3} times one fp32
scale, and quant_relu() (with the given act scales) yields integer
activations in {0..15}.  Both are exactly representable in fp8e4m3, so
layers 2-4 run as exact integer arithmetic in fp8 with DoubleRow perf mode
(fp32 PSUM accumulation is exact: |partial sums| < 2^19).  Layer 1 streams
the continuous fp32 input as an fp16 hi+lo split (two fp16 matmuls
accumulating into the same PSUM bank), giving fp32-level precision at the
fp16 matmul rate.  Per-layer affine (weight scale x BN fold / act scale) is
applied on eviction: ACT does relu(z*alpha+beta), DVE does min(.,15) and
round-to-nearest-even via the +-2^23 trick, casting to fp8 for the next
layer.  All activations stay resident in SBUF between layers.

Layout: activations are kept feature-major [128, K/128, M] so each layer's
PSUM output tile ([h_tile partitions] x [batch free dim]) is directly the
next layer's contraction operand - no transposes anywhere on device.
"""

import os
import sys

import numpy as np

# The Bass kernel runs through jax/PJRT on the neuron (axon) backend. If the
# caller pinned JAX_PLATFORMS=cpu (common for running the pure-jax reference)
# and jax has not been imported yet, lift the pin so the devices are visible.
if os.environ.get("JAX_PLATFORMS") == "cpu" and "jax" not in sys.modules:
    os.environ["JAX_PLATFORMS"] = ""

B, D, H, C = 8192, 4096, 8192, 1000
NCORES = 8
M = B // NCORES          # 1024 batch rows per core
P = 128
CPAD = 1024              # padded output features (w4 zero-padded 1000->1024)
C23 = float(2.0 ** 23)   # RNE integer-rounding constant for fp32

f32 = np.float32


def _quant_int(w):
    """Integer weight codes + scale, replicating quant_weight() in fp32."""
    w = np.ascontiguousarray(w, dtype=f32)
    scale = (np.max(np.abs(w)) / f32(3.0)).astype(f32)
    q = np.round(np.clip((w / scale).astype(f32), f32(-3.0), f32(3.0))).astype(f32)
    return q, scale


def _feat_major(a, ksub):
    """[K, N] -> [128, ksub, N] with k = ks*128 + p."""
    K, N = a.shape
    assert K == ksub * P
    return np.ascontiguousarray(a.reshape(ksub, P, N).transpose(1, 0, 2))


def _w_prep(q, ksub, htiles, wdt):
    """q [Hout, K] -> [128, htiles, ksub, 128]: w[p, ht, ks, hh] = q[ht*128+hh, ks*128+p]."""
    Hout, K = q.shape
    assert Hout == htiles * P and K == ksub * P
    t = q.T.reshape(ksub, P, htiles, P).transpose(1, 2, 0, 3)
    return np.ascontiguousarray(t).astype(wdt)


def _per_part(v, ntiles):
    """[ntiles*128] -> [128, ntiles] with v[p, t] = v[t*128+p]."""
    return np.ascontiguousarray(v.reshape(ntiles, P).T, dtype=f32)


def _build_bass():
    import concourse.bacc as bacc
    import concourse.tile as tile
    from concourse import mybir
    from contextlib import ExitStack

    fp16 = mybir.dt.float16
    fp8 = mybir.dt.float8e4
    fp32 = mybir.dt.float32
    DR = mybir.MatmulPerfMode.DoubleRow
    Relu = mybir.ActivationFunctionType.Relu
    op = mybir.AluOpType

    KS1 = D // P          # 32  k-subtiles for layer 1
    KS = H // P           # 64  k-subtiles for layers 2-4
    HT = H // P           # 64  h-tiles for layers 1-3
    HT4 = CPAD // P       # 8   h-tiles for layer 4

    nc = bacc.Bacc(None, target_bir_lowering=False)

    xh_d = nc.dram_tensor("xh", [P, KS1, M], fp16, kind="ExternalInput")
    xl_d = nc.dram_tensor("xl", [P, KS1, M], fp16, kind="ExternalInput")
    w1_d = nc.dram_tensor("w1q", [P, HT, KS1, P], fp16, kind="ExternalInput")
    w2_d = nc.dram_tensor("w2q", [P, HT, KS, P], fp8, kind="ExternalInput")
    w3_d = nc.dram_tensor("w3q", [P, HT, KS, P], fp8, kind="ExternalInput")
    w4_d = nc.dram_tensor("w4q", [P, HT4, KS, P], fp8, kind="ExternalInput")
    ab_d = {}
    for i in (1, 2, 3):
        ab_d[f"al{i}"] = nc.dram_tensor(f"al{i}", [P, HT], fp32, kind="ExternalInput")
        ab_d[f"be{i}"] = nc.dram_tensor(f"be{i}", [P, HT], fp32, kind="ExternalInput")
    ab_d["al4"] = nc.dram_tensor("al4", [P, HT4], fp32, kind="ExternalInput")
    out_d = nc.dram_tensor("out", [P, HT4, M], fp32, kind="ExternalOutput")

    with tile.TileContext(nc) as tc, ExitStack() as ctx:
        const = ctx.enter_context(tc.tile_pool(name="const", bufs=1))
        acts = ctx.enter_context(tc.tile_pool(name="acts", bufs=1))
        wp = ctx.enter_context(tc.tile_pool(name="wp", bufs=4))
        pp = ctx.enter_context(tc.tile_pool(name="pp", bufs=6, space="PSUM"))
        tp = ctx.enter_context(tc.tile_pool(name="tp", bufs=4))
        ost = ctx.enter_context(tc.tile_pool(name="ost", bufs=2))

        # PE warmup: ~100 tiny matmuls on zeroed SBUF keep the tensor engine
        # busy (and un-throttle the HAM clock gate) while the first input
        # DMAs are in flight, so the real matmuls start warm at 2.4 GHz.
        wdum = const.tile([P, P], fp16, name="wdum")
        rdum = const.tile([P, P], fp16, name="rdum")
        nc.vector.memset(wdum, 0.0)
        nc.vector.memset(rdum, 0.0)
        warm_ps = pp.tile([P, P], fp32, tag="warm", name="warm_ps", bufs=1)
        for wi in range(264):
            nc.tensor.matmul(warm_ps[:, :], wdum[:, :], rdum[:, :],
                             start=True, stop=True)

        # first weight tile DMA issues before anything else on the sync queue
        w1t0 = wp.tile([P, KS1, P], fp16, tag="wt", name="w1t0")
        nc.sync.dma_start(out=w1t0, in_=w1_d[:, 0, :, :])

        ab = {}
        for name, d in ab_d.items():
            t = const.tile(list(d.shape), fp32, name=f"c_{name}")
            nc.sync.dma_start(out=t, in_=d[:])
            ab[name] = t

        def evict_quant(psum, al, be, ht, dst):
            """dst[:, :] = fp8(round(clip(relu(psum*al+be), 0, 15)))"""
            u = tp.tile([P, 512], fp32, tag="u", name="u")
            nc.scalar.activation(u, psum[:, :], Relu, bias=be, scale=al)
            v = tp.tile([P, 512], fp32, tag="v", name="v")
            nc.vector.tensor_scalar(v, u, 15.0, C23, op0=op.min, op1=op.add)
            nc.vector.tensor_scalar(dst, v, C23, None, op0=op.subtract)

        # ---- Layer 1: z1 = x @ q1.T via fp16 hi+lo, M in two halves ----
        a1 = acts.tile([P, HT, M], fp8, tag="bufC", name="a1")
        for mh in range(2):
            ms = slice(mh * 512, (mh + 1) * 512)
            xh_t = acts.tile([P, KS1, 512], fp16, tag="bufA", name=f"xh_{mh}")
            xl_t = acts.tile([P, KS1, 512], fp16, tag="bufB", name=f"xl_{mh}")
            # chunked loads across two HWDGE queues: range-based overlap
            # tracking lets the first h-tile's matmuls start once the first
            # ks chunk lands
            chunks = [(0, 4), (4, 4), (8, 8), (16, 8), (24, 8)]
            for kc, kn in chunks:
                nc.sync.dma_start(out=xh_t[:, kc:kc + kn, :],
                                  in_=xh_d[:, kc:kc + kn, ms])
            for kc, kn in chunks:
                nc.sync.dma_start(out=xl_t[:, kc:kc + kn, :],
                                  in_=xl_d[:, kc:kc + kn, ms])
            for ht in range(HT):
                if mh == 0 and ht == 0:
                    wt = w1t0
                else:
                    wt = wp.tile([P, KS1, P], fp16, tag="wt", name=f"w1_{mh}_{ht}")
                    nc.sync.dma_start(out=wt, in_=w1_d[:, ht, :, :])
                ps = pp.tile([P, 512], fp32, tag="ps", name=f"ps1_{mh}_{ht}")
                if ht == 0:
                    # first h-tile right after the x load starts: sweep the
                    # hi pass first so the PE only waits on xh, with xl
                    # still streaming in behind it
                    for ks in range(KS1):
                        nc.tensor.matmul(ps[:, :], wt[:, ks, :], xh_t[:, ks, :],
                                         start=(ks == 0), stop=False)
                    for ks in range(KS1):
                        nc.tensor.matmul(ps[:, :], wt[:, ks, :], xl_t[:, ks, :],
                                         start=False, stop=(ks == KS1 - 1))
                else:
                    for ks in range(KS1):
                        nc.tensor.matmul(ps[:, :], wt[:, ks, :], xh_t[:, ks, :],
                                         start=(ks == 0), stop=False)
                        nc.tensor.matmul(ps[:, :], wt[:, ks, :], xl_t[:, ks, :],
                                         start=False, stop=(ks == KS1 - 1))
                evict_quant(ps, ab["al1"][:, ht:ht + 1], ab["be1"][:, ht:ht + 1],
                            ht, a1[:, ht, ms])

        # ---- Layers 2-3: fp8 DoubleRow, output split across two 32KB tiles ----
        def mid_layer(idx, rhs_parts, w_d, al, be):
            # rhs_parts: list of SBUF tiles [P, 32, M] covering ks 0..63
            outs = [acts.tile([P, KS // 2, M], fp8, tag=t, name=f"a{idx}{t}")
                    for t in ("bufA", "bufB")]
            for ht in range(HT):
                wt = wp.tile([P, KS, P], fp8, tag="wt", name=f"w{idx}_{ht}")
                nc.sync.dma_start(out=wt, in_=w_d[:, ht, :, :])
                pss = [pp.tile([P, 512], fp32, tag="ps", name=f"ps{idx}_{ht}_{i}")
                       for i in range(2)]
                for ki, ks in enumerate(range(0, KS, 2)):
                    part, kk = rhs_parts[ks // 32], ks % 32
                    for mb in range(2):
                        nc.tensor.matmul(
                            pss[mb][:, :], wt[:, ks:ks + 2, :],
                            part[:, kk:kk + 2, mb * 512:(mb + 1) * 512],
                            start=(ki == 0), stop=(ki == KS // 2 - 1),
                            perf_mode=DR)
                dst = outs[ht // 32]
                for mb in range(2):
                    evict_quant(pss[mb], al[:, ht:ht + 1], be[:, ht:ht + 1],
                                ht, dst[:, ht % 32, mb * 512:(mb + 1) * 512])
            return outs

        a2 = mid_layer(2, [a1[:, :KS // 2, :], a1[:, KS // 2:, :]],
                       w2_d, ab["al2"], ab["be2"])
        a3 = acts.tile([P, HT, M], fp8, tag="bufC", name="a3")
        for ht in range(HT):
            wt = wp.tile([P, KS, P], fp8, tag="wt", name=f"w3_{ht}")
            nc.sync.dma_start(out=wt, in_=w3_d[:, ht, :, :])
            pss = [pp.tile([P, 512], fp32, tag="ps", name=f"ps3_{ht}_{i}")
                   for i in range(2)]
            for ki, ks in enumerate(range(0, KS, 2)):
                part, kk = a2[ks // 32], ks % 32
                for mb in range(2):
                    nc.tensor.matmul(
                        pss[mb][:, :], wt[:, ks:ks + 2, :],
                        part[:, kk:kk + 2, mb * 512:(mb + 1) * 512],
                        start=(ki == 0), stop=(ki == KS // 2 - 1),
                        perf_mode=DR)
            for mb in range(2):
                evict_quant(pss[mb], ab["al3"][:, ht:ht + 1], ab["be3"][:, ht:ht + 1],
                            ht, a3[:, ht, mb * 512:(mb + 1) * 512])

        # ---- Layer 4: out = (a3 @ q4.T) * (s4 * act_scale3) ----
        for ht in range(HT4):
            wt = wp.tile([P, KS, P], fp8, tag="wt", name=f"w4_{ht}")
            nc.sync.dma_start(out=wt, in_=w4_d[:, ht, :, :])
            pss = [pp.tile([P, 512], fp32, tag="ps", name=f"ps4_{ht}_{i}")
                   for i in range(2)]
            for ki, ks in enumerate(range(0, KS, 2)):
                for mb in range(2):
                    nc.tensor.matmul(
                        pss[mb][:, :], wt[:, ks:ks + 2, :],
                        a3[:, ks:ks + 2, mb * 512:(mb + 1) * 512],
                        start=(ki == 0), stop=(ki == KS // 2 - 1),
                        perf_mode=DR)
            ot = ost.tile([P, M], fp32, tag="ot", name=f"ot_{ht}")
            for mb in range(2):
                ms = slice(mb * 512, (mb + 1) * 512)
                nc.vector.tensor_scalar(ot[:, ms], pss[mb],
                                        ab["al4"][:, ht:ht + 1], None, op0=op.mult)
                nc.sync.dma_start(out=out_d[:, ht, ms], in_=ot[:, ms])

    nc.compile()
    return nc


_NC_CACHE = {}


def kernel(x, w1, w2, w3, w4, bn_scale1, bn_bias1, bn_scale2, bn_bias2,
           bn_scale3, bn_bias3, act_scale1, act_scale2, act_scale3,
           _trace=False, _tmpdir=None):
    from concourse import mybir
    from concourse.bass_utils import run_bass_kernel_spmd

    np16 = mybir.dt.np(mybir.dt.float16)
    np8 = mybir.dt.np(mybir.dt.float8e4)

    # ---- host-side prep (exact fp32 replication of the quantizers) ----
    q1, s1 = _quant_int(w1)
    q2, s2 = _quant_int(w2)
    q3, s3 = _quant_int(w3)
    q4, s4 = _quant_int(w4)

    as1 = f32(np.asarray(act_scale1).reshape(-1)[0])
    as2 = f32(np.asarray(act_scale2).reshape(-1)[0])
    as3 = f32(np.asarray(act_scale3).reshape(-1)[0])

    al1 = (s1 * np.asarray(bn_scale1, f32) / as1).astype(f32)
    be1 = (np.asarray(bn_bias1, f32) / as1).astype(f32)
    al2 = (s2 * as1 * np.asarray(bn_scale2, f32) / as2).astype(f32)
    be2 = (np.asarray(bn_bias2, f32) / as2).astype(f32)
    al3 = (s3 * as2 * np.asarray(bn_scale3, f32) / as3).astype(f32)
    be3 = (np.asarray(bn_bias3, f32) / as3).astype(f32)
    al4 = np.full((CPAD,), s4 * as3, f32)

    q4p = np.zeros((CPAD, H), f32)
    q4p[:C] = q4

    w1p = _w_prep(q1, D // P, H // P, np16)
    w2p = _w_prep(q2, H // P, H // P, np8)
    w3p = _w_prep(q3, H // P, H // P, np8)
    w4p = _w_prep(q4p, H // P, CPAD // P, np8)

    x = np.asarray(x, f32)
    x_hi = x.astype(np16)
    x_lo = (x - x_hi.astype(f32)).astype(np16)

    shared = {
        "w1q": w1p, "w2q": w2p, "w3q": w3p, "w4q": w4p,
        "al1": _per_part(al1, H // P), "be1": _per_part(be1, H // P),
        "al2": _per_part(al2, H // P), "be2": _per_part(be2, H // P),
        "al3": _per_part(al3, H // P), "be3": _per_part(be3, H // P),
        "al4": _per_part(al4, CPAD // P),
    }
    in_maps = []
    for c in range(NCORES):
        rows = slice(c * M, (c + 1) * M)
        in_maps.append({
            **shared,
            "xh": _feat_major(x_hi[rows].T.astype(f32), D // P).astype(np16),
            "xl": _feat_major(x_lo[rows].T.astype(f32), D // P).astype(np16),
        })

    if "nc" not in _NC_CACHE:
        _NC_CACHE["nc"] = _build_bass()
    nc = _NC_CACHE["nc"]

    res = run_bass_kernel_spmd(nc, in_maps, core_ids=list(range(NCORES)),
                               trace=_trace, tmpdir=_tmpdir)
    outs = []
    for c in range(NCORES):
        o = np.asarray(res.results[c]["out"])          # [P, HT4, M]
        z = o.transpose(1, 0, 2).reshape(CPAD, M)[:C]  # [1000, M]
        outs.append(z.T)                               # [M, 1000]
    full = np.concatenate(outs, axis=0).astype(f32)
    if _trace:
        return full, res
    return full



# revision 8
# speedup vs baseline: 1.1326x; 1.1326x over previous
"""Trainium2 Bass kernel for the 4-layer quantized MLP (dense_mlp).

Strategy
--------
Data-parallel over the batch dim: each of the 8 cores processes 1024 of the
8192 rows with the full set of weights (no collectives; host gathers).

Numerics: quant_weight() yields integer codes q in {-3..3} times one fp32
scale, and quant_relu() (with the given act scales) yields integer
activations in {0..15}.  Both are exactly representable in fp8e4m3, so
layers 2-4 run as exact integer arithmetic in fp8 with DoubleRow perf mode
(fp32 PSUM accumulation is exact: |partial sums| < 2^19).  Layer 1 splits
the continuous fp32 input into xh = fp16(x) plus an fp8 residual plane
u1 = e4m3((x - xh) * 2^12): the hi pass uses fp16 weights q*2^12 (exact)
so PSUM accumulates 2^12*h1, and the lo pass adds u1 against e4m3 weights
q via DoubleRow at 2x rate; the 2^-12 is folded into the eviction scale.
That gives ~17 bits of x mantissa (sim rel err 1.66e-2 vs 2e-2 gate) at
1.5 fp16-pass cost instead of the 2.0 of an fp16 hi+lo split.  Per-layer
affine (weight scale x BN fold / act scale) is applied on eviction: ACT
does relu(z*alpha+beta), DVE does min(.,15) and round-to-nearest-even via
the +-2^23 trick, casting to fp8 for the next layer.  All activations stay
resident in SBUF between layers.

Layout: activations are kept feature-major [128, K/128, M] so each layer's
PSUM output tile ([h_tile partitions] x [batch free dim]) is directly the
next layer's contraction operand - no transposes anywhere on device.
"""

import os
import sys

import numpy as np

# The Bass kernel runs through jax/PJRT on the neuron (axon) backend. If the
# caller pinned JAX_PLATFORMS=cpu (common for running the pure-jax reference)
# and jax has not been imported yet, lift the pin so the devices are visible.
if os.environ.get("JAX_PLATFORMS") == "cpu" and "jax" not in sys.modules:
    os.environ["JAX_PLATFORMS"] = ""

B, D, H, C = 8192, 4096, 8192, 1000
NCORES = 8
M = B // NCORES          # 1024 batch rows per core
P = 128
CPAD = 1024              # padded output features (w4 zero-padded 1000->1024)
C23 = float(2.0 ** 23)   # RNE integer-rounding constant for fp32
S12 = float(2.0 ** 12)   # layer-1 residual-plane scale (PSUM holds 2^12*h1)

f32 = np.float32


def _quant_int(w):
    """Integer weight codes + scale, replicating quant_weight() in fp32."""
    w = np.ascontiguousarray(w, dtype=f32)
    scale = (np.max(np.abs(w)) / f32(3.0)).astype(f32)
    q = np.round(np.clip((w / scale).astype(f32), f32(-3.0), f32(3.0))).astype(f32)
    return q, scale


def _feat_major(a, ksub):
    """[K, N] -> [128, ksub, N] with k = ks*128 + p."""
    K, N = a.shape
    assert K == ksub * P
    return np.ascontiguousarray(a.reshape(ksub, P, N).transpose(1, 0, 2))


def _w_prep(q, ksub, htiles, wdt):
    """q [Hout, K] -> [128, htiles, ksub, 128]: w[p, ht, ks, hh] = q[ht*128+hh, ks*128+p]."""
    Hout, K = q.shape
    assert Hout == htiles * P and K == ksub * P
    t = q.T.reshape(ksub, P, htiles, P).transpose(1, 2, 0, 3)
    return np.ascontiguousarray(t).astype(wdt)


def _per_part(v, ntiles):
    """[ntiles*128] -> [128, ntiles] with v[p, t] = v[t*128+p]."""
    return np.ascontiguousarray(v.reshape(ntiles, P).T, dtype=f32)


def _build_bass():
    import concourse.bacc as bacc
    import concourse.tile as tile
    from concourse import mybir
    from contextlib import ExitStack

    fp16 = mybir.dt.float16
    fp8 = mybir.dt.float8e4
    fp32 = mybir.dt.float32
    DR = mybir.MatmulPerfMode.DoubleRow
    Relu = mybir.ActivationFunctionType.Relu
    op = mybir.AluOpType

    KS1 = D // P          # 32  k-subtiles for layer 1
    KS = H // P           # 64  k-subtiles for layers 2-4
    HT = H // P           # 64  h-tiles for layers 1-3
    HT4 = CPAD // P       # 8   h-tiles for layer 4

    nc = bacc.Bacc(None, target_bir_lowering=False)

    xh_d = nc.dram_tensor("xh", [P, KS1, M], fp16, kind="ExternalInput")
    xu_d = nc.dram_tensor("xu", [P, KS1, M], fp8, kind="ExternalInput")
    w1_d = nc.dram_tensor("w1q", [P, HT, KS1, P], fp16, kind="ExternalInput")
    w1u_d = nc.dram_tensor("w1u", [P, HT, KS1, P], fp8, kind="ExternalInput")
    w2_d = nc.dram_tensor("w2q", [P, HT, KS, P], fp8, kind="ExternalInput")
    w3_d = nc.dram_tensor("w3q", [P, HT, KS, P], fp8, kind="ExternalInput")
    w4_d = nc.dram_tensor("w4q", [P, HT4, KS, P], fp8, kind="ExternalInput")
    ab_d = {}
    for i in (1, 2, 3):
        ab_d[f"al{i}"] = nc.dram_tensor(f"al{i}", [P, HT], fp32, kind="ExternalInput")
        ab_d[f"be{i}"] = nc.dram_tensor(f"be{i}", [P, HT], fp32, kind="ExternalInput")
    ab_d["al4"] = nc.dram_tensor("al4", [P, HT4], fp32, kind="ExternalInput")
    out_d = nc.dram_tensor("out", [P, HT4, M], fp32, kind="ExternalOutput")

    with tile.TileContext(nc) as tc, ExitStack() as ctx:
        const = ctx.enter_context(tc.tile_pool(name="const", bufs=1))
        acts = ctx.enter_context(tc.tile_pool(name="acts", bufs=1))
        wp = ctx.enter_context(tc.tile_pool(name="wp", bufs=4))
        pp = ctx.enter_context(tc.tile_pool(name="pp", bufs=6, space="PSUM"))
        tp = ctx.enter_context(tc.tile_pool(name="tp", bufs=4))
        ost = ctx.enter_context(tc.tile_pool(name="ost", bufs=2))

        # PE warmup: ~100 tiny matmuls on zeroed SBUF keep the tensor engine
        # busy (and un-throttle the HAM clock gate) while the first input
        # DMAs are in flight, so the real matmuls start warm at 2.4 GHz.
        wdum = const.tile([P, P], fp16, name="wdum")
        rdum = const.tile([P, P], fp16, name="rdum")
        nc.vector.memset(wdum, 0.0)
        nc.vector.memset(rdum, 0.0)
        warm_ps = pp.tile([P, P], fp32, tag="warm", name="warm_ps", bufs=1)
        for wi in range(264):
            nc.tensor.matmul(warm_ps[:, :], wdum[:, :], rdum[:, :],
                             start=True, stop=True)

        # first weight tile DMA issues before anything else on the sync queue
        w1t0 = wp.tile([P, KS1, P], fp16, tag="wt", name="w1t0")
        nc.sync.dma_start(out=w1t0, in_=w1_d[:, 0, :, :])

        ab = {}
        for name, d in ab_d.items():
            t = const.tile(list(d.shape), fp32, name=f"c_{name}")
            nc.sync.dma_start(out=t, in_=d[:])
            ab[name] = t

        def evict_quant(psum, al, be, ht, dst):
            """dst[:, :] = fp8(round(clip(relu(psum*al+be), 0, 15)))"""
            u = tp.tile([P, 512], fp32, tag="u", name="u")
            nc.scalar.activation(u, psum[:, :], Relu, bias=be, scale=al)
            v = tp.tile([P, 512], fp32, tag="v", name="v")
            nc.vector.tensor_scalar(v, u, 15.0, C23, op0=op.min, op1=op.add)
            nc.vector.tensor_scalar(dst, v, C23, None, op0=op.subtract)

        # ---- Layer 1: PSUM accumulates 2^12*h1 = xh @ (q1*2^12).T (fp16)
        # ----          plus u1 @ q1.T (fp8 DoubleRow), M in two halves ----
        a1 = acts.tile([P, HT, M], fp8, tag="bufC", name="a1")
        for mh in range(2):
            ms = slice(mh * 512, (mh + 1) * 512)
            xh_t = acts.tile([P, KS1, 512], fp16, tag="bufA", name=f"xh_{mh}")
            xu_t = acts.tile([P, KS1, 512], fp8, tag="bufB", name=f"xu_{mh}")
            # chunked loads across two HWDGE queues: range-based overlap
            # tracking lets the first h-tile's matmuls start once the first
            # ks chunk lands
            chunks = [(0, 4), (4, 4), (8, 8), (16, 8), (24, 8)]
            for kc, kn in chunks:
                nc.sync.dma_start(out=xh_t[:, kc:kc + kn, :],
                                  in_=xh_d[:, kc:kc + kn, ms])
            for kc, kn in chunks:
                nc.sync.dma_start(out=xu_t[:, kc:kc + kn, :],
                                  in_=xu_d[:, kc:kc + kn, ms])
            for ht in range(HT):
                if mh == 0 and ht == 0:
                    wt = w1t0
                else:
                    wt = wp.tile([P, KS1, P], fp16, tag="wt", name=f"w1_{mh}_{ht}")
                    nc.sync.dma_start(out=wt, in_=w1_d[:, ht, :, :])
                wu = wp.tile([P, KS1, P], fp8, tag="wt", name=f"w1u_{mh}_{ht}")
                nc.sync.dma_start(out=wu, in_=w1u_d[:, ht, :, :])
                ps = pp.tile([P, 512], fp32, tag="ps", name=f"ps1_{mh}_{ht}")
                # hi sweep first so the PE only waits on xh, with the fp8
                # residual plane still streaming in behind it
                for ks in range(KS1):
                    nc.tensor.matmul(ps[:, :], wt[:, ks, :], xh_t[:, ks, :],
                                     start=(ks == 0), stop=False)
                for ks in range(0, KS1, 2):
                    nc.tensor.matmul(ps[:, :], wu[:, ks:ks + 2, :],
                                     xu_t[:, ks:ks + 2, :],
                                     start=False, stop=(ks == KS1 - 2),
                                     perf_mode=DR)
                evict_quant(ps, ab["al1"][:, ht:ht + 1], ab["be1"][:, ht:ht + 1],
                            ht, a1[:, ht, ms])

        # ---- Layers 2-3: fp8 DoubleRow, output split across two 32KB tiles ----
        def mid_layer(idx, rhs_parts, w_d, al, be):
            # rhs_parts: list of SBUF tiles [P, 32, M] covering ks 0..63
            outs = [acts.tile([P, KS // 2, M], fp8, tag=t, name=f"a{idx}{t}")
                    for t in ("bufA", "bufB")]
            for ht in range(HT):
                wt = wp.tile([P, KS, P], fp8, tag="wt", name=f"w{idx}_{ht}")
                nc.sync.dma_start(out=wt, in_=w_d[:, ht, :, :])
                pss = [pp.tile([P, 512], fp32, tag="ps", name=f"ps{idx}_{ht}_{i}")
                       for i in range(2)]
                for ki, ks in enumerate(range(0, KS, 2)):
                    part, kk = rhs_parts[ks // 32], ks % 32
                    for mb in range(2):
                        nc.tensor.matmul(
                            pss[mb][:, :], wt[:, ks:ks + 2, :],
                            part[:, kk:kk + 2, mb * 512:(mb + 1) * 512],
                            start=(ki == 0), stop=(ki == KS // 2 - 1),
                            perf_mode=DR)
                dst = outs[ht // 32]
                for mb in range(2):
                    evict_quant(pss[mb], al[:, ht:ht + 1], be[:, ht:ht + 1],
                                ht, dst[:, ht % 32, mb * 512:(mb + 1) * 512])
            return outs

        a2 = mid_layer(2, [a1[:, :KS // 2, :], a1[:, KS // 2:, :]],
                       w2_d, ab["al2"], ab["be2"])
        a3 = acts.tile([P, HT, M], fp8, tag="bufC", name="a3")
        for ht in range(HT):
            wt = wp.tile([P, KS, P], fp8, tag="wt", name=f"w3_{ht}")
            nc.sync.dma_start(out=wt, in_=w3_d[:, ht, :, :])
            pss = [pp.tile([P, 512], fp32, tag="ps", name=f"ps3_{ht}_{i}")
                   for i in range(2)]
            for ki, ks in enumerate(range(0, KS, 2)):
                part, kk = a2[ks // 32], ks % 32
                for mb in range(2):
                    nc.tensor.matmul(
                        pss[mb][:, :], wt[:, ks:ks + 2, :],
                        part[:, kk:kk + 2, mb * 512:(mb + 1) * 512],
                        start=(ki == 0), stop=(ki == KS // 2 - 1),
                        perf_mode=DR)
            for mb in range(2):
                evict_quant(pss[mb], ab["al3"][:, ht:ht + 1], ab["be3"][:, ht:ht + 1],
                            ht, a3[:, ht, mb * 512:(mb + 1) * 512])

        # ---- Layer 4: out = (a3 @ q4.T) * (s4 * act_scale3) ----
        for ht in range(HT4):
            wt = wp.tile([P, KS, P], fp8, tag="wt", name=f"w4_{ht}")
            nc.sync.dma_start(out=wt, in_=w4_d[:, ht, :, :])
            pss = [pp.tile([P, 512], fp32, tag="ps", name=f"ps4_{ht}_{i}")
                   for i in range(2)]
            for ki, ks in enumerate(range(0, KS, 2)):
                for mb in range(2):
                    nc.tensor.matmul(
                        pss[mb][:, :], wt[:, ks:ks + 2, :],
                        a3[:, ks:ks + 2, mb * 512:(mb + 1) * 512],
                        start=(ki == 0), stop=(ki == KS // 2 - 1),
                        perf_mode=DR)
            ot = ost.tile([P, M], fp32, tag="ot", name=f"ot_{ht}")
            for mb in range(2):
                ms = slice(mb * 512, (mb + 1) * 512)
                nc.vector.tensor_scalar(ot[:, ms], pss[mb],
                                        ab["al4"][:, ht:ht + 1], None, op0=op.mult)
                nc.sync.dma_start(out=out_d[:, ht, ms], in_=ot[:, ms])

    nc.compile()
    return nc


_NC_CACHE = {}


def kernel(x, w1, w2, w3, w4, bn_scale1, bn_bias1, bn_scale2, bn_bias2,
           bn_scale3, bn_bias3, act_scale1, act_scale2, act_scale3,
           _trace=False, _tmpdir=None):
    from concourse import mybir
    from concourse.bass_utils import run_bass_kernel_spmd

    np16 = mybir.dt.np(mybir.dt.float16)
    np8 = mybir.dt.np(mybir.dt.float8e4)

    # ---- host-side prep (exact fp32 replication of the quantizers) ----
    q1, s1 = _quant_int(w1)
    q2, s2 = _quant_int(w2)
    q3, s3 = _quant_int(w3)
    q4, s4 = _quant_int(w4)

    as1 = f32(np.asarray(act_scale1).reshape(-1)[0])
    as2 = f32(np.asarray(act_scale2).reshape(-1)[0])
    as3 = f32(np.asarray(act_scale3).reshape(-1)[0])

    # PSUM holds 2^12*h1 for layer 1 (weights q1*2^12) -> fold 2^-12 here
    al1 = (s1 * np.asarray(bn_scale1, f32) / (as1 * f32(S12))).astype(f32)
    be1 = (np.asarray(bn_bias1, f32) / as1).astype(f32)
    al2 = (s2 * as1 * np.asarray(bn_scale2, f32) / as2).astype(f32)
    be2 = (np.asarray(bn_bias2, f32) / as2).astype(f32)
    al3 = (s3 * as2 * np.asarray(bn_scale3, f32) / as3).astype(f32)
    be3 = (np.asarray(bn_bias3, f32) / as3).astype(f32)
    al4 = np.full((CPAD,), s4 * as3, f32)

    q4p = np.zeros((CPAD, H), f32)
    q4p[:C] = q4

    w1p = _w_prep(q1 * f32(S12), D // P, H // P, np16)   # q1*2^12 exact in fp16
    w1u = _w_prep(q1, D // P, H // P, np8)
    w2p = _w_prep(q2, H // P, H // P, np8)
    w3p = _w_prep(q3, H // P, H // P, np8)
    w4p = _w_prep(q4p, H // P, CPAD // P, np8)

    x = np.asarray(x, f32)
    x_hi = x.astype(np16)
    x_u1 = ((x - x_hi.astype(f32)) * f32(S12)).astype(np8)

    shared = {
        "w1q": w1p, "w1u": w1u, "w2q": w2p, "w3q": w3p, "w4q": w4p,
        "al1": _per_part(al1, H // P), "be1": _per_part(be1, H // P),
        "al2": _per_part(al2, H // P), "be2": _per_part(be2, H // P),
        "al3": _per_part(al3, H // P), "be3": _per_part(be3, H // P),
        "al4": _per_part(al4, CPAD // P),
    }
    in_maps = []
    for c in range(NCORES):
        rows = slice(c * M, (c + 1) * M)
        in_maps.append({
            **shared,
            "xh": _feat_major(x_hi[rows].T.astype(f32), D // P).astype(np16),
            "xu": _feat_major(x_u1[rows].T.astype(f32), D // P).astype(np8),
        })

    if "nc" not in _NC_CACHE:
        _NC_CACHE["nc"] = _build_bass()
    nc = _NC_CACHE["nc"]

    res = run_bass_kernel_spmd(nc, in_maps, core_ids=list(range(NCORES)),
                               trace=_trace, tmpdir=_tmpdir)
    outs = []
    for c in range(NCORES):
        o = np.asarray(res.results[c]["out"])          # [P, HT4, M]
        z = o.transpose(1, 0, 2).reshape(CPAD, M)[:C]  # [1000, M]
        outs.append(z.T)                               # [M, 1000]
    full = np.concatenate(outs, axis=0).astype(f32)
    if _trace:
        return full, res
    return full



# revision 12
# speedup vs baseline: 1.1377x; 1.0045x over previous
"""Trainium2 Bass kernel for the 4-layer quantized MLP (dense_mlp).

Strategy
--------
Data-parallel over the batch dim: each of the 8 cores processes 1024 of the
8192 rows with the full set of weights (no collectives; host gathers).

Numerics: quant_weight() yields integer codes q in {-3..3} times one fp32
scale, and quant_relu() (with the given act scales) yields integer
activations in {0..15}.  Both are exactly representable in fp8e4m3, so
layers 2-4 run as exact integer arithmetic in fp8 with DoubleRow perf mode
(fp32 PSUM accumulation is exact: |partial sums| < 2^19).  Layer 1 splits
the continuous fp32 input into xh = fp16(x) plus an fp8 residual plane
u1 = e4m3((x - xh) * 2^12): the hi pass uses fp16 weights q*2^12 (exact)
so PSUM accumulates 2^12*h1, and the lo pass adds u1 against e4m3 weights
q via DoubleRow at 2x rate; the 2^-12 is folded into the eviction scale.
That gives ~17 bits of x mantissa (sim rel err 1.66e-2 vs 2e-2 gate) at
1.5 fp16-pass cost instead of the 2.0 of an fp16 hi+lo split.  Per-layer
affine (weight scale x BN fold / act scale) is applied on eviction: ACT
does relu(z*alpha+beta), DVE does min(.,15) and round-to-nearest-even via
the +-2^23 trick, casting to fp8 for the next layer.  All activations stay
resident in SBUF between layers.

Layout: activations are kept feature-major [128, K/128, M] so each layer's
PSUM output tile ([h_tile partitions] x [batch free dim]) is directly the
next layer's contraction operand - no transposes anywhere on device.
"""

import os
import sys

import numpy as np

# The Bass kernel runs through jax/PJRT on the neuron (axon) backend. If the
# caller pinned JAX_PLATFORMS=cpu (common for running the pure-jax reference)
# and jax has not been imported yet, lift the pin so the devices are visible.
if os.environ.get("JAX_PLATFORMS") == "cpu" and "jax" not in sys.modules:
    os.environ["JAX_PLATFORMS"] = ""

B, D, H, C = 8192, 4096, 8192, 1000
NCORES = 8
M = B // NCORES          # 1024 batch rows per core
P = 128
CPAD = 1024              # padded output features (w4 zero-padded 1000->1024)
C23 = float(2.0 ** 23)   # RNE integer-rounding constant for fp32
S12 = float(2.0 ** 12)   # layer-1 residual-plane scale (PSUM holds 2^12*h1)

f32 = np.float32


def _quant_int(w):
    """Integer weight codes + scale, replicating quant_weight() in fp32."""
    w = np.ascontiguousarray(w, dtype=f32)
    scale = (np.max(np.abs(w)) / f32(3.0)).astype(f32)
    q = np.round(np.clip((w / scale).astype(f32), f32(-3.0), f32(3.0))).astype(f32)
    return q, scale


def _feat_major(a, ksub):
    """[K, N] -> [128, ksub, N] with k = ks*128 + p."""
    K, N = a.shape
    assert K == ksub * P
    return np.ascontiguousarray(a.reshape(ksub, P, N).transpose(1, 0, 2))


def _w_prep(q, ksub, htiles, wdt):
    """q [Hout, K] -> [128, htiles, ksub, 128]: w[p, ht, ks, hh] = q[ht*128+hh, ks*128+p]."""
    Hout, K = q.shape
    assert Hout == htiles * P and K == ksub * P
    t = q.T.reshape(ksub, P, htiles, P).transpose(1, 2, 0, 3)
    return np.ascontiguousarray(t).astype(wdt)


def _per_part(v, ntiles):
    """[ntiles*128] -> [128, ntiles] with v[p, t] = v[t*128+p]."""
    return np.ascontiguousarray(v.reshape(ntiles, P).T, dtype=f32)


def _build_bass():
    import concourse.bacc as bacc
    import concourse.tile as tile
    from concourse import mybir
    from contextlib import ExitStack

    fp16 = mybir.dt.float16
    fp8 = mybir.dt.float8e4
    fp32 = mybir.dt.float32
    DR = mybir.MatmulPerfMode.DoubleRow
    Relu = mybir.ActivationFunctionType.Relu
    op = mybir.AluOpType

    KS1 = D // P          # 32  k-subtiles for layer 1
    KS = H // P           # 64  k-subtiles for layers 2-4
    HT = H // P           # 64  h-tiles for layers 1-3
    HT4 = CPAD // P       # 8   h-tiles for layer 4

    nc = bacc.Bacc(None, target_bir_lowering=False)

    xh_d = nc.dram_tensor("xh", [P, KS1, M], fp16, kind="ExternalInput")
    xu_d = nc.dram_tensor("xu", [P, KS1, M], fp8, kind="ExternalInput")
    w1_d = nc.dram_tensor("w1q", [P, HT, KS1, P], fp16, kind="ExternalInput")
    w1u_d = nc.dram_tensor("w1u", [P, HT, KS1, P], fp8, kind="ExternalInput")
    w2_d = nc.dram_tensor("w2q", [P, HT, KS, P], fp8, kind="ExternalInput")
    w3_d = nc.dram_tensor("w3q", [P, HT, KS, P], fp8, kind="ExternalInput")
    w4_d = nc.dram_tensor("w4q", [P, HT4, KS, P], fp8, kind="ExternalInput")
    ab_d = {}
    for i in (1, 2, 3):
        ab_d[f"al{i}"] = nc.dram_tensor(f"al{i}", [P, HT], fp32, kind="ExternalInput")
        ab_d[f"be{i}"] = nc.dram_tensor(f"be{i}", [P, HT], fp32, kind="ExternalInput")
    ab_d["al4"] = nc.dram_tensor("al4", [P, HT4], fp32, kind="ExternalInput")
    out_d = nc.dram_tensor("out", [P, HT4, M], fp32, kind="ExternalOutput")

    with tile.TileContext(nc) as tc, ExitStack() as ctx:
        const = ctx.enter_context(tc.tile_pool(name="const", bufs=1))
        acts = ctx.enter_context(tc.tile_pool(name="acts", bufs=1))
        wp = ctx.enter_context(tc.tile_pool(name="wp", bufs=4))
        pp = ctx.enter_context(tc.tile_pool(name="pp", bufs=6, space="PSUM"))
        tp = ctx.enter_context(tc.tile_pool(name="tp", bufs=2))
        ost = ctx.enter_context(tc.tile_pool(name="ost", bufs=1))

        # PE warmup: tiny matmuls on zeroed SBUF keep the tensor engine
        # busy (and un-throttle the HAM clock gate) while the first input
        # DMAs are in flight, so the real matmuls start warm at 2.4 GHz.
        wdum = const.tile([P, P], fp16, name="wdum")
        rdum = const.tile([P, P], fp16, name="rdum")
        nc.vector.memset(wdum, 0.0)
        nc.vector.memset(rdum, 0.0)
        warm_ps = pp.tile([P, P], fp32, tag="warm", name="warm_ps", bufs=1)
        for wi in range(160):
            nc.tensor.matmul(warm_ps[:, :], wdum[:, :], rdum[:, :],
                             start=True, stop=True)

        # first weight tile DMA issues before anything else on the sync queue
        w1t0 = wp.tile([P, KS1, P], fp16, tag="wt", name="w1t0")
        nc.sync.dma_start(out=w1t0, in_=w1_d[:, 0, :, :])

        # constants ride the second HWDGE queue (scalar/ACT engine)
        ab = {}
        for name, d in ab_d.items():
            t = const.tile(list(d.shape), fp32, name=f"c_{name}")
            nc.scalar.dma_start(out=t, in_=d[:])
            ab[name] = t

        def evict_quant(psum, al, be, ht, dst):
            """dst[:, :] = fp8(round(clip(relu(psum*al+be), 0, 15)))"""
            u = tp.tile([P, 512], fp32, tag="u", name="u")
            nc.scalar.activation(u, psum[:, :], Relu, bias=be, scale=al)
            v = tp.tile([P, 512], fp32, tag="v", name="v")
            nc.vector.tensor_scalar(v, u, 15.0, C23, op0=op.min, op1=op.add)
            nc.vector.tensor_scalar(dst, v, C23, None, op0=op.subtract)

        # ---- Layer 1: PSUM accumulates 2^12*h1 = xh @ (q1*2^12).T (fp16)
        # ----          plus u1 @ q1.T (fp8 DoubleRow), M in two halves ----
        a1 = acts.tile([P, HT, M], fp8, tag="bufC", name="a1")
        for mh in range(2):
            ms = slice(mh * 512, (mh + 1) * 512)
            # double-buffered x tiles (bufs=2): mh=1 loads prefetch during
            # mh=0 compute.  xh rides the sync HWDGE queue, xu the scalar
            # one, halving the time until the first h-tile is runnable.
            xh_t = acts.tile([P, KS1, 512], fp16, tag="xh2", bufs=2,
                             name=f"xh_{mh}")
            xu_t = acts.tile([P, KS1, 512], fp8, tag="xu2", bufs=2,
                             name=f"xu_{mh}")
            # chunked loads: range-based overlap tracking lets the first
            # h-tile's matmuls start once the first ks chunk lands
            chunks = [(0, 4), (4, 4), (8, 8), (16, 8), (24, 8)]
            for kc, kn in chunks:
                nc.sync.dma_start(out=xh_t[:, kc:kc + kn, :],
                                  in_=xh_d[:, kc:kc + kn, ms])
            for kc, kn in chunks:
                nc.scalar.dma_start(out=xu_t[:, kc:kc + kn, :],
                                    in_=xu_d[:, kc:kc + kn, ms])
            for ht in range(HT):
                if mh == 0 and ht == 0:
                    wt = w1t0
                else:
                    wt = wp.tile([P, KS1, P], fp16, tag="wt", name=f"w1_{mh}_{ht}")
                    nc.sync.dma_start(out=wt, in_=w1_d[:, ht, :, :])
                wu = wp.tile([P, KS1, P], fp8, tag="wt", name=f"w1u_{mh}_{ht}")
                nc.scalar.dma_start(out=wu, in_=w1u_d[:, ht, :, :])
                ps = pp.tile([P, 512], fp32, tag="ps", name=f"ps1_{mh}_{ht}")
                # hi sweep first so the PE only waits on xh, with the fp8
                # residual plane still streaming in behind it
                for ks in range(KS1):
                    nc.tensor.matmul(ps[:, :], wt[:, ks, :], xh_t[:, ks, :],
                                     start=(ks == 0), stop=False)
                for ks in range(0, KS1, 2):
                    nc.tensor.matmul(ps[:, :], wu[:, ks:ks + 2, :],
                                     xu_t[:, ks:ks + 2, :],
                                     start=False, stop=(ks == KS1 - 2),
                                     perf_mode=DR)
                evict_quant(ps, ab["al1"][:, ht:ht + 1], ab["be1"][:, ht:ht + 1],
                            ht, a1[:, ht, ms])

        # ---- Layers 2-3: fp8 DoubleRow, output split across two 32KB tiles ----
        def mid_layer(idx, rhs_parts, w_d, al, be):
            # rhs_parts: list of SBUF tiles [P, 32, M] covering ks 0..63
            # (reuses the two 32KB xh2 slots; xh is dead once layer 1 ends)
            outs = [acts.tile([P, KS // 2, M], fp8, tag="xh2", bufs=2,
                              name=f"a{idx}_{i}") for i in range(2)]
            for ht in range(HT):
                wt = wp.tile([P, KS, P], fp8, tag="wt", name=f"w{idx}_{ht}")
                nc.sync.dma_start(out=wt, in_=w_d[:, ht, :, :])
                pss = [pp.tile([P, 512], fp32, tag="ps", name=f"ps{idx}_{ht}_{i}")
                       for i in range(2)]
                for ki, ks in enumerate(range(0, KS, 2)):
                    part, kk = rhs_parts[ks // 32], ks % 32
                    for mb in range(2):
                        nc.tensor.matmul(
                            pss[mb][:, :], wt[:, ks:ks + 2, :],
                            part[:, kk:kk + 2, mb * 512:(mb + 1) * 512],
                            start=(ki == 0), stop=(ki == KS // 2 - 1),
                            perf_mode=DR)
                dst = outs[ht // 32]
                for mb in range(2):
                    evict_quant(pss[mb], al[:, ht:ht + 1], be[:, ht:ht + 1],
                                ht, dst[:, ht % 32, mb * 512:(mb + 1) * 512])
            return outs

        a2 = mid_layer(2, [a1[:, :KS // 2, :], a1[:, KS // 2:, :]],
                       w2_d, ab["al2"], ab["be2"])
        a3 = acts.tile([P, HT, M], fp8, tag="bufC", name="a3")
        for ht in range(HT):
            wt = wp.tile([P, KS, P], fp8, tag="wt", name=f"w3_{ht}")
            nc.sync.dma_start(out=wt, in_=w3_d[:, ht, :, :])
            pss = [pp.tile([P, 512], fp32, tag="ps", name=f"ps3_{ht}_{i}")
                   for i in range(2)]
            for ki, ks in enumerate(range(0, KS, 2)):
                part, kk = a2[ks // 32], ks % 32
                for mb in range(2):
                    nc.tensor.matmul(
                        pss[mb][:, :], wt[:, ks:ks + 2, :],
                        part[:, kk:kk + 2, mb * 512:(mb + 1) * 512],
                        start=(ki == 0), stop=(ki == KS // 2 - 1),
                        perf_mode=DR)
            for mb in range(2):
                evict_quant(pss[mb], ab["al3"][:, ht:ht + 1], ab["be3"][:, ht:ht + 1],
                            ht, a3[:, ht, mb * 512:(mb + 1) * 512])

        # ---- Layer 4: out = (a3 @ q4.T) * (s4 * act_scale3) ----
        for ht in range(HT4):
            wt = wp.tile([P, KS, P], fp8, tag="wt", name=f"w4_{ht}")
            nc.sync.dma_start(out=wt, in_=w4_d[:, ht, :, :])
            ot = ost.tile([P, M], fp32, tag="ot", name=f"ot_{ht}")
            if ht < HT4 - 1:
                pss = [pp.tile([P, 512], fp32, tag="ps", name=f"ps4_{ht}_{i}")
                       for i in range(2)]
                for ki, ks in enumerate(range(0, KS, 2)):
                    for mb in range(2):
                        nc.tensor.matmul(
                            pss[mb][:, :], wt[:, ks:ks + 2, :],
                            a3[:, ks:ks + 2, mb * 512:(mb + 1) * 512],
                            start=(ki == 0), stop=(ki == KS // 2 - 1),
                            perf_mode=DR)
                for mb in range(2):
                    ms = slice(mb * 512, (mb + 1) * 512)
                    nc.vector.tensor_scalar(ot[:, ms], pss[mb],
                                            ab["al4"][:, ht:ht + 1], None,
                                            op0=op.mult)
                    nc.sync.dma_start(out=out_d[:, ht, ms], in_=ot[:, ms])
            else:
                # last h-tile: run the two batch halves sequentially so the
                # first half's eviction+DMA hides under the second's matmuls
                for mb in range(2):
                    ms = slice(mb * 512, (mb + 1) * 512)
                    ps = pp.tile([P, 512], fp32, tag="ps", name=f"ps4_{ht}_{mb}")
                    for ki, ks in enumerate(range(0, KS, 2)):
                        nc.tensor.matmul(
                            ps[:, :], wt[:, ks:ks + 2, :],
                            a3[:, ks:ks + 2, ms],
                            start=(ki == 0), stop=(ki == KS // 2 - 1),
                            perf_mode=DR)
                    nc.vector.tensor_scalar(ot[:, ms], ps,
                                            ab["al4"][:, ht:ht + 1], None,
                                            op0=op.mult)
                    nc.sync.dma_start(out=out_d[:, ht, ms], in_=ot[:, ms])

    nc.compile()
    return nc


_NC_CACHE = {}


def kernel(x, w1, w2, w3, w4, bn_scale1, bn_bias1, bn_scale2, bn_bias2,
           bn_scale3, bn_bias3, act_scale1, act_scale2, act_scale3,
           _trace=False, _tmpdir=None):
    from concourse import mybir
    from concourse.bass_utils import run_bass_kernel_spmd

    np16 = mybir.dt.np(mybir.dt.float16)
    np8 = mybir.dt.np(mybir.dt.float8e4)

    # ---- host-side prep (exact fp32 replication of the quantizers) ----
    q1, s1 = _quant_int(w1)
    q2, s2 = _quant_int(w2)
    q3, s3 = _quant_int(w3)
    q4, s4 = _quant_int(w4)

    as1 = f32(np.asarray(act_scale1).reshape(-1)[0])
    as2 = f32(np.asarray(act_scale2).reshape(-1)[0])
    as3 = f32(np.asarray(act_scale3).reshape(-1)[0])

    # PSUM holds 2^12*h1 for layer 1 (weights q1*2^12) -> fold 2^-12 here
    al1 = (s1 * np.asarray(bn_scale1, f32) / (as1 * f32(S12))).astype(f32)
    be1 = (np.asarray(bn_bias1, f32) / as1).astype(f32)
    al2 = (s2 * as1 * np.asarray(bn_scale2, f32) / as2).astype(f32)
    be2 = (np.asarray(bn_bias2, f32) / as2).astype(f32)
    al3 = (s3 * as2 * np.asarray(bn_scale3, f32) / as3).astype(f32)
    be3 = (np.asarray(bn_bias3, f32) / as3).astype(f32)
    al4 = np.full((CPAD,), s4 * as3, f32)

    q4p = np.zeros((CPAD, H), f32)
    q4p[:C] = q4

    w1p = _w_prep(q1 * f32(S12), D // P, H // P, np16)   # q1*2^12 exact in fp16
    w1u = _w_prep(q1, D // P, H // P, np8)
    w2p = _w_prep(q2, H // P, H // P, np8)
    w3p = _w_prep(q3, H // P, H // P, np8)
    w4p = _w_prep(q4p, H // P, CPAD // P, np8)

    x = np.asarray(x, f32)
    x_hi = x.astype(np16)
    x_u1 = ((x - x_hi.astype(f32)) * f32(S12)).astype(np8)

    shared = {
        "w1q": w1p, "w1u": w1u, "w2q": w2p, "w3q": w3p, "w4q": w4p,
        "al1": _per_part(al1, H // P), "be1": _per_part(be1, H // P),
        "al2": _per_part(al2, H // P), "be2": _per_part(be2, H // P),
        "al3": _per_part(al3, H // P), "be3": _per_part(be3, H // P),
        "al4": _per_part(al4, CPAD // P),
    }
    in_maps = []
    for c in range(NCORES):
        rows = slice(c * M, (c + 1) * M)
        in_maps.append({
            **shared,
            "xh": _feat_major(x_hi[rows].T.astype(f32), D // P).astype(np16),
            "xu": _feat_major(x_u1[rows].T.astype(f32), D // P).astype(np8),
        })

    if "nc" not in _NC_CACHE:
        _NC_CACHE["nc"] = _build_bass()
    nc = _NC_CACHE["nc"]

    res = run_bass_kernel_spmd(nc, in_maps, core_ids=list(range(NCORES)),
                               trace=_trace, tmpdir=_tmpdir)
    outs = []
    for c in range(NCORES):
        o = np.asarray(res.results[c]["out"])          # [P, HT4, M]
        z = o.transpose(1, 0, 2).reshape(CPAD, M)[:C]  # [1000, M]
        outs.append(z.T)                               # [M, 1000]
    full = np.concatenate(outs, axis=0).astype(f32)
    if _trace:
        return full, res
    return full

